# revision 23
# baseline (speedup 1.0000x reference)
"""Distributed causal multi-head attention for Trainium2 (8 NeuronCores).

Problem: B=2, S=2048, NX=1024, H=16 heads, D=64.
  qkv = x @ w_attn + b_attn ; q,k,v split; causal softmax(q k^T / 8) v ; @ w_proj + b_proj

Sharding: core c -> batch b=c//4 (data parallel), head group g=c%4 (tensor
parallel, 4 heads). Column-split c_attn; after attention two AllToAlls
reshard heads->sequence so each core computes c_proj for its own 512 output
rows with the full hidden dim - no cross-core reduction.

Layout strategy: host passes x transposed (xT [NX, S]) so QKV projections,
scores and PV products all run in matmul-native layouts with zero on-chip
transposes. Scores are computed transposed ([k, q]): the softmax reduction
over k lands on the partition axis, where an extra ones-column appended to V
yields the denominator for free in the same PV matmul. exp() needs no
max-subtraction (scores are bounded; ACT exp is <=2 ULP on [-10,10]).
Attention is key-block-major with up-to-1024-wide score tiles so the ScalarE
exp runs in few wide calls. Matmul operands are bf16 (fast weight loads keep
the PE dense); accumulation stays fp32 in PSUM.
"""

import sys

sys.path.insert(0, "/opt/trn_rl_repo")

import numpy as np
import ml_dtypes

BF16 = ml_dtypes.bfloat16

B = 2
S = 2048
NX = 1024
H = 16
D = 64
G = 4            # head groups (tensor-parallel)
HL = H // G      # heads per core = 4
HDW = HL * D     # head-group width = 256
P = 128
SC = 512         # output chunk (A2A granularity)
NQC = S // SC    # 4 chunks
NE = NX // P     # 8 contraction tiles
NKB = S // P     # 16 key blocks
WQ = 1024        # max score-tile width

_COMPILED = None


def _build():
    import concourse.bass as bass  # noqa: F401
    import concourse.mybir as mybir
    import concourse.tile as tile
    from concourse import bacc

    f32 = mybir.dt.float32
    f32r = mybir.dt.float32r
    bf16 = mybir.dt.bfloat16
    Identity = mybir.ActivationFunctionType.Identity
    Exp = mybir.ActivationFunctionType.Exp

    nc = bacc.Bacc("TRN2", target_bir_lowering=False, debug=False, num_devices=8)

    xT = nc.dram_tensor("xT", [NX, S], bf16, kind="ExternalInput")
    wqk = nc.dram_tensor("wqk", [NX, 2 * HDW], bf16, kind="ExternalInput")
    wv = nc.dram_tensor("wv", [NX, HDW], bf16, kind="ExternalInput")
    wp = nc.dram_tensor("wp", [2 * NX, NX], bf16, kind="ExternalInput")
    bqk = nc.dram_tensor("bqk", [4, P], f32, kind="ExternalInput")
    bv = nc.dram_tensor("bv", [1, HDW], f32, kind="ExternalInput")
    bp16 = nc.dram_tensor("bp16", [1, NX], bf16, kind="ExternalInput")
    causalT = nc.dram_tensor("causalT", [P, P], f32, kind="ExternalInput")
    onesc = nc.dram_tensor("onesc", [P, 4], f32, kind="ExternalInput")
    onesb = nc.dram_tensor("onesb", [P, 4], bf16, kind="ExternalInput")
    out_ext = nc.dram_tensor("out", [SC, NX], f32, kind="ExternalOutput")

    with tile.TileContext(nc) as tc:
        with (
            tc.tile_pool(name="const", bufs=1) as const_pool,
            tc.tile_pool(name="xt", bufs=1) as xt_pool,
            tc.tile_pool(name="w", bufs=1) as w_pool,
            tc.tile_pool(name="qkt", bufs=1) as qkt_pool,
            tc.tile_pool(name="vsb", bufs=1) as v_pool,
            tc.tile_pool(name="lh", bufs=1) as lh_pool,
            tc.tile_pool(name="exp", bufs=3) as exp_pool,
            tc.tile_pool(name="osb", bufs=2) as osb_pool,
            tc.tile_pool(name="small", bufs=2) as small_pool,
            tc.tile_pool(name="wide", bufs=2, space="PSUM") as wide_ps,
            tc.tile_pool(name="atps", bufs=4, space="PSUM") as at_ps_pool,
            tc.tile_pool(name="dram", bufs=1, space="DRAM") as dram_pool,
        ):
            # ---- constants ----
            bqk_sb = const_pool.tile([P, 4], f32, name="bqk_sb")
            for fi in range(4):
                nc.sync.dma_start(bqk_sb[:, fi : fi + 1], bqk[fi : fi + 1, :])
            bv_sb = const_pool.tile([1, HDW], f32r, name="bv_sb")
            nc.sync.dma_start(bv_sb[:], bv[:].bitcast(f32r))
            bp_sb = const_pool.tile([1, NX], bf16, name="bp_sb")
            nc.sync.dma_start(bp_sb[:], bp16[:])
            cz_sb = const_pool.tile([P, P], f32, name="cz_sb")
            nc.sync.dma_start(cz_sb[:], causalT[:])
            ones1 = const_pool.tile([1, P], f32r, name="ones1")
            nc.sync.dma_start(ones1[:], onesc[:, 0:1].bitcast(f32r))
            ones1b = const_pool.tile([1, P], bf16, name="ones1b")
            nc.sync.dma_start(ones1b[:], onesb[:, 0:1])

            # ---- weight + xT loads (first-needed first) ----
            wqk_sb = []
            xt_sb = {}
            for e in range(NE):
                t = w_pool.tile([P, 2 * HDW], bf16, name=f"wqk_sb{e}")
                nc.sync.dma_start(t[:], wqk[e * P : (e + 1) * P, :])
                wqk_sb.append(t)
                t2 = xt_pool.tile([P, SC], bf16, name=f"xt{e}_0", tag=f"xts{e}_0")
                nc.sync.dma_start(t2[:], xT[e * P : (e + 1) * P, 0:SC])
                xt_sb[e, 0] = t2
            wv_sb = []
            for e in range(NE):
                t = w_pool.tile([P, HDW], bf16, name=f"wv_sb{e}")
                nc.sync.dma_start(t[:], wv[e * P : (e + 1) * P, :])
                wv_sb.append(t)
            for sc in range(1, NQC):
                for e in range(NE):
                    t = xt_pool.tile([P, SC], bf16, name=f"xt{e}_{sc}", tag=f"xts{e}_{sc}")
                    nc.sync.dma_start(
                        t[:], xT[e * P : (e + 1) * P, sc * SC : (sc + 1) * SC]
                    )
                    xt_sb[e, sc] = t

            # extended (junk-masked) w_proj tiles, loaded late into recycled
            # xT slots (xt slot (e, sc) frees once phase 1 consumed it)
            wp_sb = {}
            for kt2 in range(2 * NE):
                for nn2 in range(2):
                    t = xt_pool.tile(
                        [P, SC], bf16, name=f"wp{kt2}_{nn2}",
                        tag=f"xts{kt2 % NE}_{(kt2 // NE) * 2 + nn2}",
                    )
                    nc.sync.dma_start(
                        t[:], wp[kt2 * P : (kt2 + 1) * P, nn2 * SC : (nn2 + 1) * SC]
                    )
                    wp_sb[kt2, nn2] = t

            # ---- phase 1: qkT [2*HDW, S] (full-S tiles) and v [S, padded] ----
            qkt_sb = {}
            v_sb = {}
            for fi in range(4):
                qkt_sb[fi] = qkt_pool.tile(
                    [P, S], bf16, name=f"qkt{fi}", tag=f"qktw{fi}"
                )
            for sc in range(NQC):
                for fi in range(4):
                    ps = wide_ps.tile([P, SC], f32, tag="wide", name=f"qk_ps{fi}_{sc}")
                    for e in range(NE):
                        nc.tensor.matmul(
                            ps[:],
                            wqk_sb[e][:, fi * P : (fi + 1) * P],
                            xt_sb[e, sc][:],
                            start=(e == 0),
                            stop=(e == NE - 1),
                        )
                    # fold the 1/sqrt(D)=1/8 score scale into q (bias comes
                    # pre-scaled from the host)
                    scale = 0.125 if fi < 2 else 1.0
                    nc.scalar.activation(
                        qkt_sb[fi][:, sc * SC : (sc + 1) * SC],
                        ps[:],
                        Identity,
                        bias=bqk_sb[:, fi : fi + 1],
                        scale=scale,
                    )
                for j in range(4):
                    si = 4 * sc + j
                    psv = wide_ps.tile([P, HDW], f32, tag="wide", name=f"v_ps{si}")
                    for e in range(NE):
                        nc.tensor.matmul(
                            psv[:],
                            xt_sb[e, sc][:, j * P : (j + 1) * P],
                            wv_sb[e][:],
                            start=(e == 0),
                            stop=(e == NE - 1),
                        )
                    # per-head 128-wide slots: [v(64) | ones(1) | zeros(63)]
                    vt = v_pool.tile([P, HL * P], bf16, name=f"v{si}")
                    nc.vector.memset(vt[:], 0.0)
                    nc.sync.dma_start(
                        vt[:].rearrange("p (h u) -> p h u", h=HL)[:, :, D : D + 1],
                        onesb[:],
                    )
                    nc.scalar.activation(
                        vt[:].rearrange("p (h u) -> p h u", h=HL)[:, :, 0:D],
                        psv[:].rearrange("p (h u) -> p h u", h=HL),
                        Identity,
                    )
                    v_sb[si] = vt

            # ---- phase 2: attention, key-block-major per head ----
            # A2A per head-pair kt: chunk j carries heads (2kt, 2kt+1) for
            # s-range j%4; the receiver zeroes the other batch's chunks via wp.
            a2a_in = {}
            a2a_out = {}
            for kt in range(2):
                a2a_in[kt] = dram_pool.tile(
                    [8, P, SC], bf16, tag=f"a2a_in{kt}", name=f"a2a_in{kt}"
                )
                a2a_out[kt] = dram_pool.tile(
                    [8, P, SC], bf16, tag=f"a2a_out{kt}", name=f"a2a_out{kt}"
                )

            def emit_tail(h, qc, at_ps):
                dn_r = small_pool.tile([1, SC], f32r, tag="dnr", name=f"dnr{qc}_{h}")
                nc.vector.tensor_copy(dn_r[:], at_ps[D : D + 1, :])
                # bv folded in as a rank-1 update: (PV + bv denom^T)/denom
                nc.tensor.matmul(
                    at_ps[0:D, :],
                    bv_sb[0:1, h * D : (h + 1) * D],
                    dn_r[:],
                    start=False,
                    stop=True,
                )
                dn32 = small_pool.tile([1, SC], f32, tag="dn32", name=f"dn32{qc}_{h}")
                nc.vector.tensor_copy(dn32[:], at_ps[D : D + 1, :])
                rc32 = small_pool.tile([1, SC], f32, tag="rc32", name=f"rc32{qc}_{h}")
                nc.vector.reciprocal_approx_fast(rc32[:], dn32[:])
                rc = small_pool.tile([1, SC], f32r, tag="rc", name=f"rc{qc}_{h}")
                nc.vector.tensor_copy(rc[:], rc32[:])
                # broadcast 1/denom across partitions via K=1 matmul
                rb = wide_ps.tile([D, SC], f32, tag="wide", name=f"rb{qc}_{h}")
                nc.tensor.matmul(rb[:], ones1[0:1, 0:D], rc[:], start=True, stop=True)
                rb_sb = small_pool.tile([D, SC], f32, tag="rbsb", name=f"rbsb{qc}_{h}")
                nc.vector.tensor_copy(rb_sb[:], rb[:])
                ath = small_pool.tile(
                    [D, SC], bf16, tag="ath", bufs=3, name=f"ath{qc}_{h}"
                )
                nc.vector.tensor_mul(ath[:], at_ps[0:D, :], rb_sb[:])
                kt = h // 2
                prow = 64 * (h % 2)
                nc.sync.dma_start(a2a_in[kt][qc, prow : prow + D, :], ath[:])
                nc.sync.dma_start(a2a_in[kt][qc + 4, prow : prow + D, :], ath[:])

            for h in range(HL):
                fi_q = h // 2
                fi_k = 2 + h // 2
                prow = 64 * (h % 2)
                at_ps = {}
                done = set()
                for qc in range(NQC):
                    at_ps[qc] = at_ps_pool.tile(
                        [P, SC], f32, tag="atps", name=f"at_ps{qc}_{h}"
                    )
                for kb in range(NKB):
                    q0 = P * kb
                    width = S - q0
                    for s2 in range((width + WQ - 1) // WQ):
                        w0 = q0 + s2 * WQ
                        ww = min(WQ, S - w0)
                        scp = wide_ps.tile(
                            [P, WQ], f32, tag="wide", name=f"sc{h}_{kb}_{s2}"
                        )
                        # scoresT[k, q] = kT^T qT (contraction over d);
                        # matmul output is capped at one PSUM bank (512 f32)
                        for m0 in range(0, ww, SC):
                            mw = min(SC, ww - m0)
                            nc.tensor.matmul(
                                scp[:, m0 : m0 + mw],
                                qkt_sb[fi_k][prow : prow + D, q0 : q0 + P],
                                qkt_sb[fi_q][prow : prow + D, w0 + m0 : w0 + m0 + mw],
                                start=True,
                                stop=True,
                            )
                        if s2 == 0:
                            # diagonal block: additive causal mask
                            nc.vector.tensor_add(scp[:, 0:P], scp[:, 0:P], cz_sb[:])
                        ex = exp_pool.tile(
                            [P, WQ], bf16, tag="exp", name=f"ex{h}_{kb}_{s2}"
                        )
                        nc.scalar.activation(ex[:, 0:ww], scp[:, 0:ww], Exp)
                        # PV pieces per output chunk qc (+denominator via the
                        # ones column of v)
                        qc_lo = w0 // SC
                        qc_hi = (w0 + ww - 1) // SC
                        for qc in range(qc_lo, qc_hi + 1):
                            a0 = max(w0, qc * SC)
                            a1 = min(w0 + ww, (qc + 1) * SC)
                            nc.tensor.matmul(
                                at_ps[qc][:, a0 - qc * SC : a1 - qc * SC],
                                v_sb[kb][:, h * P : (h + 1) * P],
                                ex[:, a0 - w0 : a1 - w0],
                                start=(kb == 0),
                                stop=False,
                            )
                    # deferred normalization: ~2 key-blocks after a chunk's
                    # last contribution, so the PE rarely waits on the DVE
                    # reciprocal chain
                    for qc in range(NQC):
                        if kb == min(4 * qc + 3 + 2, NKB - 1) and qc not in done:
                            done.add(qc)
                            emit_tail(h, qc, at_ps[qc])
                if h % 2 == 1:
                    kt = h // 2
                    nc.gpsimd.collective_compute(
                        "AllToAll",
                        mybir.AluOpType.bypass,
                        ins=[a2a_in[kt][:].opt()],
                        outs=[a2a_out[kt][:].opt()],
                        replica_groups=[list(range(8))],
                    )

            # ---- phase 3: gather + c_proj ----
            # extended-hd row 128*kt2 .. +128 = source core j = kt2//2,
            # head pair kt = kt2%2; other batch's chunks are masked by wp zeros
            lh_sb = {}
            for kt2 in range(2 * NE):
                t = lh_pool.tile([P, SC], bf16, name=f"lh{kt2}")
                nc.sync.dma_start(t[:], a2a_out[kt2 % 2][kt2 // 2])
                lh_sb[kt2] = t
            for st in range(4):
                o_sb = osb_pool.tile([P, NX], f32, tag="osb", name=f"osb{st}")
                for nn2 in range(2):
                    pp = wide_ps.tile([P, SC], f32, tag="wide", name=f"pj_ps{st}_{nn2}")
                    for kt2 in range(2 * NE):
                        nc.tensor.matmul(
                            pp[:],
                            lh_sb[kt2][:, st * P : (st + 1) * P],
                            wp_sb[kt2, nn2][:],
                            start=(kt2 == 0),
                            stop=False,
                        )
                    # + b_proj via K=1 ones matmul (each core owns its rows)
                    nc.tensor.matmul(
                        pp[:],
                        ones1b[0:1, :],
                        bp_sb[0:1, nn2 * SC : (nn2 + 1) * SC],
                        start=False,
                        stop=True,
                    )
                    nc.vector.tensor_copy(o_sb[:, nn2 * SC : (nn2 + 1) * SC], pp[:])
                nc.sync.dma_start(out_ext[st * P : (st + 1) * P, :], o_sb[:])

    nc.compile()
    return nc


def _get_compiled():
    global _COMPILED
    if _COMPILED is None:
        _COMPILED = _build()
    return _COMPILED


def make_in_maps(x, attention_mask, w_attn, b_attn, w_proj, b_proj):
    x = np.asarray(x, dtype=np.float32)
    w_attn = np.asarray(w_attn, dtype=np.float32)
    b_attn = np.asarray(b_attn, dtype=np.float32)
    w_proj = np.asarray(w_proj, dtype=np.float32)
    b_proj = np.asarray(b_proj, dtype=np.float32)

    ki, qi = np.meshgrid(np.arange(P), np.arange(P), indexing="ij")
    causalT = np.where(ki > qi, np.float32(-1e9), np.float32(0.0))
    xTs = [np.ascontiguousarray(x[b].T.astype(BF16)) for b in range(B)]
    # extended w_proj: rows [1024*b : 1024*(b+1)] hold the real w_proj, the
    # other batch's rows are zero (masks that batch's A2A chunks)
    wp_ext = []
    for b in range(B):
        w = np.zeros((2 * NX, NX), dtype=BF16)
        w[NX * b : NX * (b + 1), :] = w_proj.astype(BF16)
        wp_ext.append(w)
    bp_row16 = np.ascontiguousarray(b_proj.reshape(1, NX).astype(BF16))

    in_maps = []
    for c in range(8):
        b, g = divmod(c, 4)
        cols = slice(HDW * g, HDW * (g + 1))
        kcols = slice(NX + HDW * g, NX + HDW * (g + 1))
        vcols = slice(2 * NX + HDW * g, 2 * NX + HDW * (g + 1))
        bqk_arr = np.concatenate([b_attn[cols] * 0.125, b_attn[kcols]]).reshape(4, P)
        in_maps.append(
            {
                "xT": xTs[b],
                "wqk": np.ascontiguousarray(
                    np.concatenate([w_attn[:, cols], w_attn[:, kcols]], axis=1).astype(
                        BF16
                    )
                ),
                "wv": np.ascontiguousarray(w_attn[:, vcols].astype(BF16)),
                "wp": wp_ext[b],
                "bqk": np.ascontiguousarray(bqk_arr),
                "bv": np.ascontiguousarray(b_attn[vcols].reshape(1, HDW)),
                "bp16": bp_row16,
                "causalT": causalT,
                "onesc": np.ones((P, 4), dtype=np.float32),
                "onesb": np.ones((P, 4), dtype=BF16),
            }
        )
    return in_maps


def assemble_out(results):
    out = np.empty((B, S, NX), dtype=np.float32)
    for c in range(8):
        b, g = divmod(c, 4)
        out[b, g * SC : (g + 1) * SC, :] = results[c]["out"]
    return out


def run(in_maps, trace=False):
    from concourse.bass_utils import run_bass_kernel_spmd

    nc = _get_compiled()
    return run_bass_kernel_spmd(nc, in_maps, core_ids=list(range(8)), trace=trace)


def kernel(**inputs) -> np.ndarray:
    in_maps = make_in_maps(**inputs)
    res = run(in_maps)
    return assemble_out(res.results)


if __name__ == "__main__":
    _get_compiled()
    print("build+compile OK")


# revision 26
# speedup vs baseline: 1.0229x; 1.0229x over previous
"""Distributed causal multi-head attention for Trainium2 (8 NeuronCores).

Problem: B=2, S=2048, NX=1024, H=16 heads, D=64.
  qkv = x @ w_attn + b_attn ; q,k,v split; causal softmax(q k^T / 8) v ; @ w_proj + b_proj

Sharding: core c -> batch b=c//4 (data parallel), head group g=c%4 (tensor
parallel, 4 heads). Column-split c_attn; after attention two AllToAlls
reshard heads->sequence so each core computes c_proj for its own 512 output
rows with the full hidden dim - no cross-core reduction.

Layout strategy: host passes x transposed (xT [NX, S]) so QKV projections,
scores and PV products all run in matmul-native layouts with zero on-chip
transposes. Scores are computed transposed ([k, q]): the softmax reduction
over k lands on the partition axis, where an extra ones-column appended to V
yields the denominator for free in the same PV matmul. exp() needs no
max-subtraction (scores are bounded; ACT exp is <=2 ULP on [-10,10]).
Attention is key-block-major with up-to-1024-wide score tiles so the ScalarE
exp runs in few wide calls. Matmul operands are bf16 (fast weight loads keep
the PE dense); accumulation stays fp32 in PSUM.
"""

import sys

sys.path.insert(0, "/opt/trn_rl_repo")

import numpy as np
import ml_dtypes

BF16 = ml_dtypes.bfloat16

B = 2
S = 2048
NX = 1024
H = 16
D = 64
G = 4            # head groups (tensor-parallel)
HL = H // G      # heads per core = 4
HDW = HL * D     # head-group width = 256
P = 128
SC = 512         # output chunk (A2A granularity)
NQC = S // SC    # 4 chunks
NE = NX // P     # 8 contraction tiles
NKB = S // P     # 16 key blocks
WQ = 1024        # max score-tile width

_COMPILED = None


def _build():
    import concourse.bass as bass  # noqa: F401
    import concourse.mybir as mybir
    import concourse.tile as tile
    from concourse import bacc

    f32 = mybir.dt.float32
    f32r = mybir.dt.float32r
    bf16 = mybir.dt.bfloat16
    Identity = mybir.ActivationFunctionType.Identity
    Exp = mybir.ActivationFunctionType.Exp

    nc = bacc.Bacc("TRN2", target_bir_lowering=False, debug=False, num_devices=8)

    xT = nc.dram_tensor("xT", [NX, S], bf16, kind="ExternalInput")
    wqk = nc.dram_tensor("wqk", [NX, 2 * HDW], bf16, kind="ExternalInput")
    wv = nc.dram_tensor("wv", [NX, HDW], bf16, kind="ExternalInput")
    wp = nc.dram_tensor("wp", [2 * NX, NX], bf16, kind="ExternalInput")
    bqk = nc.dram_tensor("bqk", [4, P], f32, kind="ExternalInput")
    bv = nc.dram_tensor("bv", [1, HDW], f32, kind="ExternalInput")
    bp16 = nc.dram_tensor("bp16", [1, NX], bf16, kind="ExternalInput")
    causalT = nc.dram_tensor("causalT", [P, P], f32, kind="ExternalInput")
    onesc = nc.dram_tensor("onesc", [P, 4], f32, kind="ExternalInput")
    onesb = nc.dram_tensor("onesb", [P, 4], bf16, kind="ExternalInput")
    out_ext = nc.dram_tensor("out", [SC, NX], f32, kind="ExternalOutput")

    with tile.TileContext(nc) as tc:
        with (
            tc.tile_pool(name="const", bufs=1) as const_pool,
            tc.tile_pool(name="xt", bufs=1) as xt_pool,
            tc.tile_pool(name="w", bufs=1) as w_pool,
            tc.tile_pool(name="qkt", bufs=1) as qkt_pool,
            tc.tile_pool(name="vsb", bufs=1) as v_pool,
            tc.tile_pool(name="lh", bufs=1) as lh_pool,
            tc.tile_pool(name="exp", bufs=3) as exp_pool,
            tc.tile_pool(name="osb", bufs=2) as osb_pool,
            tc.tile_pool(name="small", bufs=2) as small_pool,
            tc.tile_pool(name="wide", bufs=2, space="PSUM") as wide_ps,
            tc.tile_pool(name="atps", bufs=4, space="PSUM") as at_ps_pool,
            tc.tile_pool(name="dram", bufs=1, space="DRAM") as dram_pool,
        ):
            # ---- constants ----
            bqk_sb = const_pool.tile([P, 4], f32, name="bqk_sb")
            for fi in range(4):
                nc.sync.dma_start(bqk_sb[:, fi : fi + 1], bqk[fi : fi + 1, :])
            bv_sb = const_pool.tile([1, HDW], f32r, name="bv_sb")
            nc.sync.dma_start(bv_sb[:], bv[:].bitcast(f32r))
            bp_sb = const_pool.tile([1, NX], bf16, name="bp_sb")
            nc.sync.dma_start(bp_sb[:], bp16[:])
            cz_sb = const_pool.tile([P, P], f32, name="cz_sb")
            nc.sync.dma_start(cz_sb[:], causalT[:])
            ones1 = const_pool.tile([1, P], f32r, name="ones1")
            nc.sync.dma_start(ones1[:], onesc[:, 0:1].bitcast(f32r))
            ones1b = const_pool.tile([1, P], bf16, name="ones1b")
            nc.sync.dma_start(ones1b[:], onesb[:, 0:1])

            # ---- weight + xT loads (first-needed first) ----
            wqk_sb = []
            xt_sb = {}
            for e in range(NE):
                t = w_pool.tile([P, 2 * HDW], bf16, name=f"wqk_sb{e}")
                nc.sync.dma_start(t[:], wqk[e * P : (e + 1) * P, :])
                wqk_sb.append(t)
                t2 = xt_pool.tile([P, SC], bf16, name=f"xt{e}_0", tag=f"xts{e}_0")
                nc.sync.dma_start(t2[:], xT[e * P : (e + 1) * P, 0:SC])
                xt_sb[e, 0] = t2
            wv_sb = []
            for e in range(NE):
                t = w_pool.tile([P, HDW], bf16, name=f"wv_sb{e}")
                nc.sync.dma_start(t[:], wv[e * P : (e + 1) * P, :])
                wv_sb.append(t)
            for sc in range(1, NQC):
                for e in range(NE):
                    t = xt_pool.tile([P, SC], bf16, name=f"xt{e}_{sc}", tag=f"xts{e}_{sc}")
                    nc.sync.dma_start(
                        t[:], xT[e * P : (e + 1) * P, sc * SC : (sc + 1) * SC]
                    )
                    xt_sb[e, sc] = t

            # extended (junk-masked) w_proj tiles, loaded late into recycled
            # xT slots (xt slot (e, sc) frees once phase 1 consumed it)
            wp_sb = {}
            for kt2 in range(2 * NE):
                for nn2 in range(2):
                    t = xt_pool.tile(
                        [P, SC], bf16, name=f"wp{kt2}_{nn2}",
                        tag=f"xts{kt2 % NE}_{(kt2 // NE) * 2 + nn2}",
                    )
                    nc.sync.dma_start(
                        t[:], wp[kt2 * P : (kt2 + 1) * P, nn2 * SC : (nn2 + 1) * SC]
                    )
                    wp_sb[kt2, nn2] = t

            # ---- phase 1: qkT [2*HDW, S] (full-S tiles) and v [S, padded] ----
            qkt_sb = {}
            v_sb = {}
            for fi in range(2):
                qkt_sb[fi] = qkt_pool.tile(
                    [P, S], bf16, name=f"qkt{fi}", tag=f"qktw{fi}"
                )
            # per-head kT with the other head's rows zeroed: score matmuls
            # then run at K=128 (zeros annihilate the foreign q rows), which
            # keeps the PE array utilization high enough for the HAM clock
            # gate to run at full rate
            ktz_sb = {}
            for h in range(HL):
                ktz_sb[h] = qkt_pool.tile([P, S], bf16, name=f"ktz{h}", tag=f"ktz{h}")
                nc.vector.memset(ktz_sb[h][:], 0.0)
            for sc in range(NQC):
                for fi in range(4):
                    ps = wide_ps.tile([P, SC], f32, tag="wide", name=f"qk_ps{fi}_{sc}")
                    for e in range(NE):
                        nc.tensor.matmul(
                            ps[:],
                            wqk_sb[e][:, fi * P : (fi + 1) * P],
                            xt_sb[e, sc][:],
                            start=(e == 0),
                            stop=(e == NE - 1),
                        )
                    # fold the 1/sqrt(D)=1/8 score scale into q (bias comes
                    # pre-scaled from the host)
                    if fi < 2:
                        nc.scalar.activation(
                            qkt_sb[fi][:, sc * SC : (sc + 1) * SC],
                            ps[:],
                            Identity,
                            bias=bqk_sb[:, fi : fi + 1],
                            scale=0.125,
                        )
                    else:
                        for hh in range(2):
                            h = 2 * (fi - 2) + hh
                            r0 = 64 * hh
                            nc.scalar.activation(
                                ktz_sb[h][r0 : r0 + D, sc * SC : (sc + 1) * SC],
                                ps[r0 : r0 + D, :],
                                Identity,
                                bias=bqk_sb[r0 : r0 + D, fi : fi + 1],
                            )
                for j in range(4):
                    si = 4 * sc + j
                    psv = wide_ps.tile([P, HDW], f32, tag="wide", name=f"v_ps{si}")
                    for e in range(NE):
                        nc.tensor.matmul(
                            psv[:],
                            xt_sb[e, sc][:, j * P : (j + 1) * P],
                            wv_sb[e][:],
                            start=(e == 0),
                            stop=(e == NE - 1),
                        )
                    # per-head 128-wide slots: [v(64) | ones(1) | zeros(63)]
                    vt = v_pool.tile([P, HL * P], bf16, name=f"v{si}")
                    nc.vector.memset(vt[:], 0.0)
                    nc.sync.dma_start(
                        vt[:].rearrange("p (h u) -> p h u", h=HL)[:, :, D : D + 1],
                        onesb[:],
                    )
                    nc.scalar.activation(
                        vt[:].rearrange("p (h u) -> p h u", h=HL)[:, :, 0:D],
                        psv[:].rearrange("p (h u) -> p h u", h=HL),
                        Identity,
                    )
                    v_sb[si] = vt

            # ---- phase 2: attention, key-block-major per head ----
            # A2A per head-pair kt: chunk j carries heads (2kt, 2kt+1) for
            # s-range j%4; the receiver zeroes the other batch's chunks via wp.
            a2a_in = {}
            a2a_out = {}
            for kt in range(2):
                a2a_in[kt] = dram_pool.tile(
                    [8, P, SC], bf16, tag=f"a2a_in{kt}", name=f"a2a_in{kt}"
                )
                a2a_out[kt] = dram_pool.tile(
                    [8, P, SC], bf16, tag=f"a2a_out{kt}", name=f"a2a_out{kt}"
                )

            def emit_tail(h, qc, at_ps):
                dn_r = small_pool.tile([1, SC], f32r, tag="dnr", name=f"dnr{qc}_{h}")
                nc.vector.tensor_copy(dn_r[:], at_ps[D : D + 1, :])
                # bv folded in as a rank-1 update: (PV + bv denom^T)/denom
                nc.tensor.matmul(
                    at_ps[0:D, :],
                    bv_sb[0:1, h * D : (h + 1) * D],
                    dn_r[:],
                    start=False,
                    stop=True,
                )
                dn32 = small_pool.tile([1, SC], f32, tag="dn32", name=f"dn32{qc}_{h}")
                nc.vector.tensor_copy(dn32[:], at_ps[D : D + 1, :])
                rc32 = small_pool.tile([1, SC], f32, tag="rc32", name=f"rc32{qc}_{h}")
                nc.vector.reciprocal_approx_fast(rc32[:], dn32[:])
                rc = small_pool.tile([1, SC], f32r, tag="rc", name=f"rc{qc}_{h}")
                nc.vector.tensor_copy(rc[:], rc32[:])
                # broadcast 1/denom across partitions via K=1 matmul
                rb = wide_ps.tile([D, SC], f32, tag="wide", name=f"rb{qc}_{h}")
                nc.tensor.matmul(rb[:], ones1[0:1, 0:D], rc[:], start=True, stop=True)
                rb_sb = small_pool.tile([D, SC], f32, tag="rbsb", name=f"rbsb{qc}_{h}")
                nc.vector.tensor_copy(rb_sb[:], rb[:])
                ath = small_pool.tile(
                    [D, SC], bf16, tag="ath", bufs=3, name=f"ath{qc}_{h}"
                )
                nc.vector.tensor_mul(ath[:], at_ps[0:D, :], rb_sb[:])
                kt = h // 2
                prow = 64 * (h % 2)
                nc.sync.dma_start(a2a_in[kt][qc, prow : prow + D, :], ath[:])
                nc.sync.dma_start(a2a_in[kt][qc + 4, prow : prow + D, :], ath[:])

            for h in range(HL):
                fi_q = h // 2
                fi_k = 2 + h // 2
                prow = 64 * (h % 2)
                at_ps = {}
                done = set()
                for qc in range(NQC):
                    at_ps[qc] = at_ps_pool.tile(
                        [P, SC], f32, tag="atps", name=f"at_ps{qc}_{h}"
                    )
                for kb in range(NKB):
                    q0 = P * kb
                    width = S - q0
                    for s2 in range((width + WQ - 1) // WQ):
                        w0 = q0 + s2 * WQ
                        ww = min(WQ, S - w0)
                        scp = wide_ps.tile(
                            [P, WQ], f32, tag="wide", name=f"sc{h}_{kb}_{s2}"
                        )
                        # scoresT[k, q] = kT^T qT (contraction over d, zero-
                        # padded to K=128); matmul output is capped at one
                        # PSUM bank (512 f32)
                        for m0 in range(0, ww, SC):
                            mw = min(SC, ww - m0)
                            nc.tensor.matmul(
                                scp[:, m0 : m0 + mw],
                                ktz_sb[h][:, q0 : q0 + P],
                                qkt_sb[fi_q][:, w0 + m0 : w0 + m0 + mw],
                                start=True,
                                stop=True,
                            )
                        if s2 == 0:
                            # diagonal block: additive causal mask
                            nc.vector.tensor_add(scp[:, 0:P], scp[:, 0:P], cz_sb[:])
                        ex = exp_pool.tile(
                            [P, WQ], bf16, tag="exp", name=f"ex{h}_{kb}_{s2}"
                        )
                        nc.scalar.activation(ex[:, 0:ww], scp[:, 0:ww], Exp)
                        # PV pieces per output chunk qc (+denominator via the
                        # ones column of v)
                        qc_lo = w0 // SC
                        qc_hi = (w0 + ww - 1) // SC
                        for qc in range(qc_lo, qc_hi + 1):
                            a0 = max(w0, qc * SC)
                            a1 = min(w0 + ww, (qc + 1) * SC)
                            nc.tensor.matmul(
                                at_ps[qc][:, a0 - qc * SC : a1 - qc * SC],
                                v_sb[kb][:, h * P : (h + 1) * P],
                                ex[:, a0 - w0 : a1 - w0],
                                start=(kb == 0),
                                stop=False,
                            )
                    # deferred normalization: ~2 key-blocks after a chunk's
                    # last contribution, so the PE rarely waits on the DVE
                    # reciprocal chain
                    for qc in range(NQC):
                        if kb == min(4 * qc + 3 + 2, NKB - 1) and qc not in done:
                            done.add(qc)
                            emit_tail(h, qc, at_ps[qc])
                if h % 2 == 1:
                    kt = h // 2
                    nc.gpsimd.collective_compute(
                        "AllToAll",
                        mybir.AluOpType.bypass,
                        ins=[a2a_in[kt][:].opt()],
                        outs=[a2a_out[kt][:].opt()],
                        replica_groups=[list(range(8))],
                    )

            # ---- phase 3: gather + c_proj ----
            # extended-hd row 128*kt2 .. +128 = source core j = kt2//2,
            # head pair kt = kt2%2; other batch's chunks are masked by wp zeros
            lh_sb = {}
            for kt2 in range(2 * NE):
                t = lh_pool.tile([P, SC], bf16, name=f"lh{kt2}")
                nc.sync.dma_start(t[:], a2a_out[kt2 % 2][kt2 // 2])
                lh_sb[kt2] = t
            for st in range(4):
                o_sb = osb_pool.tile([P, NX], f32, tag="osb", name=f"osb{st}")
                for nn2 in range(2):
                    pp = wide_ps.tile([P, SC], f32, tag="wide", name=f"pj_ps{st}_{nn2}")
                    for kt2 in range(2 * NE):
                        nc.tensor.matmul(
                            pp[:],
                            lh_sb[kt2][:, st * P : (st + 1) * P],
                            wp_sb[kt2, nn2][:],
                            start=(kt2 == 0),
                            stop=False,
                        )
                    # + b_proj via K=1 ones matmul (each core owns its rows)
                    nc.tensor.matmul(
                        pp[:],
                        ones1b[0:1, :],
                        bp_sb[0:1, nn2 * SC : (nn2 + 1) * SC],
                        start=False,
                        stop=True,
                    )
                    nc.vector.tensor_copy(o_sb[:, nn2 * SC : (nn2 + 1) * SC], pp[:])
                nc.sync.dma_start(out_ext[st * P : (st + 1) * P, :], o_sb[:])

    nc.compile()
    return nc


def _get_compiled():
    global _COMPILED
    if _COMPILED is None:
        _COMPILED = _build()
    return _COMPILED


def make_in_maps(x, attention_mask, w_attn, b_attn, w_proj, b_proj):
    x = np.asarray(x, dtype=np.float32)
    w_attn = np.asarray(w_attn, dtype=np.float32)
    b_attn = np.asarray(b_attn, dtype=np.float32)
    w_proj = np.asarray(w_proj, dtype=np.float32)
    b_proj = np.asarray(b_proj, dtype=np.float32)

    ki, qi = np.meshgrid(np.arange(P), np.arange(P), indexing="ij")
    causalT = np.where(ki > qi, np.float32(-1e9), np.float32(0.0))
    xTs = [np.ascontiguousarray(x[b].T.astype(BF16)) for b in range(B)]
    # extended w_proj: rows [1024*b : 1024*(b+1)] hold the real w_proj, the
    # other batch's rows are zero (masks that batch's A2A chunks)
    wp_ext = []
    for b in range(B):
        w = np.zeros((2 * NX, NX), dtype=BF16)
        w[NX * b : NX * (b + 1), :] = w_proj.astype(BF16)
        wp_ext.append(w)
    bp_row16 = np.ascontiguousarray(b_proj.reshape(1, NX).astype(BF16))

    in_maps = []
    for c in range(8):
        b, g = divmod(c, 4)
        cols = slice(HDW * g, HDW * (g + 1))
        kcols = slice(NX + HDW * g, NX + HDW * (g + 1))
        vcols = slice(2 * NX + HDW * g, 2 * NX + HDW * (g + 1))
        bqk_arr = np.concatenate([b_attn[cols] * 0.125, b_attn[kcols]]).reshape(4, P)
        in_maps.append(
            {
                "xT": xTs[b],
                "wqk": np.ascontiguousarray(
                    np.concatenate([w_attn[:, cols], w_attn[:, kcols]], axis=1).astype(
                        BF16
                    )
                ),
                "wv": np.ascontiguousarray(w_attn[:, vcols].astype(BF16)),
                "wp": wp_ext[b],
                "bqk": np.ascontiguousarray(bqk_arr),
                "bv": np.ascontiguousarray(b_attn[vcols].reshape(1, HDW)),
                "bp16": bp_row16,
                "causalT": causalT,
                "onesc": np.ones((P, 4), dtype=np.float32),
                "onesb": np.ones((P, 4), dtype=BF16),
            }
        )
    return in_maps


def assemble_out(results):
    out = np.empty((B, S, NX), dtype=np.float32)
    for c in range(8):
        b, g = divmod(c, 4)
        out[b, g * SC : (g + 1) * SC, :] = results[c]["out"]
    return out


def run(in_maps, trace=False):
    from concourse.bass_utils import run_bass_kernel_spmd

    nc = _get_compiled()
    return run_bass_kernel_spmd(nc, in_maps, core_ids=list(range(8)), trace=trace)


def kernel(**inputs) -> np.ndarray:
    in_maps = make_in_maps(**inputs)
    res = run(in_maps)
    return assemble_out(res.results)


if __name__ == "__main__":
    _get_compiled()
    print("build+compile OK")


# revision 27
# speedup vs baseline: 1.2427x; 1.2149x over previous
"""Distributed causal multi-head attention for Trainium2 (8 NeuronCores).

Problem: B=2, S=2048, NX=1024, H=16 heads, D=64.
  qkv = x @ w_attn + b_attn ; q,k,v split; causal softmax(q k^T / 8) v ; @ w_proj + b_proj

Sharding: core c -> batch b=c//4 (data parallel), head group g=c%4 (tensor
parallel, 4 heads). Column-split c_attn; after attention two AllToAlls
reshard heads->sequence so each core computes c_proj for its own 512 output
rows with the full hidden dim - no cross-core reduction.

Layout strategy: host passes x transposed (xT [NX, S]) so QKV projections,
scores and PV products all run in matmul-native layouts with zero on-chip
transposes. Scores are computed transposed ([k, q]): the softmax reduction
over k lands on the partition axis, where an extra ones-column appended to V
yields the denominator for free in the same PV matmul. exp() needs no
max-subtraction (scores are bounded; ACT exp is <=2 ULP on [-10,10]).
Attention is key-block-major with up-to-1024-wide score tiles so the ScalarE
exp runs in few wide calls. Matmul operands are bf16 (fast weight loads keep
the PE dense); accumulation stays fp32 in PSUM.
"""

import sys

sys.path.insert(0, "/opt/trn_rl_repo")

import numpy as np
import ml_dtypes

BF16 = ml_dtypes.bfloat16

B = 2
S = 2048
NX = 1024
H = 16
D = 64
G = 4            # head groups (tensor-parallel)
HL = H // G      # heads per core = 4
HDW = HL * D     # head-group width = 256
P = 128
SC = 512         # output chunk (A2A granularity)
NQC = S // SC    # 4 chunks
NE = NX // P     # 8 contraction tiles
NKB = S // P     # 16 key blocks
WQ = 1024        # max score-tile width

_COMPILED = None


def _build():
    import concourse.bass as bass  # noqa: F401
    import concourse.mybir as mybir
    import concourse.tile as tile
    from concourse import bacc

    f32 = mybir.dt.float32
    f32r = mybir.dt.float32r
    bf16 = mybir.dt.bfloat16
    Identity = mybir.ActivationFunctionType.Identity
    Exp = mybir.ActivationFunctionType.Exp

    nc = bacc.Bacc("TRN2", target_bir_lowering=False, debug=False, num_devices=8)

    xT = nc.dram_tensor("xT", [NX, S], bf16, kind="ExternalInput")
    wqk = nc.dram_tensor("wqk", [NX, 2 * HDW], bf16, kind="ExternalInput")
    wv = nc.dram_tensor("wv", [NX, HDW], bf16, kind="ExternalInput")
    wp = nc.dram_tensor("wp", [2 * NX, NX], bf16, kind="ExternalInput")
    bqk = nc.dram_tensor("bqk", [4, P], f32, kind="ExternalInput")
    bv = nc.dram_tensor("bv", [1, HDW], f32, kind="ExternalInput")
    bp16 = nc.dram_tensor("bp16", [1, NX], bf16, kind="ExternalInput")
    onesc = nc.dram_tensor("onesc", [P, 4], f32, kind="ExternalInput")
    onesb = nc.dram_tensor("onesb", [P, 4], bf16, kind="ExternalInput")
    identb = nc.dram_tensor("identb", [P, P], bf16, kind="ExternalInput")
    causb = nc.dram_tensor("causb", [P, P], bf16, kind="ExternalInput")
    out_ext = nc.dram_tensor("out", [SC, NX], f32, kind="ExternalOutput")

    with tile.TileContext(nc) as tc:
        with (
            tc.tile_pool(name="const", bufs=1) as const_pool,
            tc.tile_pool(name="xt", bufs=1) as xt_pool,
            tc.tile_pool(name="w", bufs=1) as w_pool,
            tc.tile_pool(name="qkt", bufs=1) as qkt_pool,
            tc.tile_pool(name="vsb", bufs=1) as v_pool,
            tc.tile_pool(name="lh", bufs=1) as lh_pool,
            tc.tile_pool(name="exp", bufs=3) as exp_pool,
            tc.tile_pool(name="osb", bufs=2) as osb_pool,
            tc.tile_pool(name="small", bufs=2) as small_pool,
            tc.tile_pool(name="wide", bufs=2, space="PSUM") as wide_ps,
            tc.tile_pool(name="atps", bufs=4, space="PSUM") as at_ps_pool,
            tc.tile_pool(name="dram", bufs=1, space="DRAM") as dram_pool,
        ):
            # ---- constants ----
            bqk_sb = const_pool.tile([P, 4], f32, name="bqk_sb")
            for fi in range(4):
                nc.sync.dma_start(bqk_sb[:, fi : fi + 1], bqk[fi : fi + 1, :])
            bv_sb = const_pool.tile([1, HDW], f32r, name="bv_sb")
            nc.sync.dma_start(bv_sb[:], bv[:].bitcast(f32r))
            bp_sb = const_pool.tile([1, NX], bf16, name="bp_sb")
            nc.sync.dma_start(bp_sb[:], bp16[:])
            id_sb = const_pool.tile([P, P], bf16, name="id_sb")
            nc.sync.dma_start(id_sb[:], identb[:])
            czb_sb = const_pool.tile([P, P], bf16, name="czb_sb")
            nc.sync.dma_start(czb_sb[:], causb[:])
            ones1 = const_pool.tile([1, P], f32r, name="ones1")
            nc.sync.dma_start(ones1[:], onesc[:, 0:1].bitcast(f32r))
            ones1b = const_pool.tile([1, P], bf16, name="ones1b")
            nc.sync.dma_start(ones1b[:], onesb[:, 0:1])

            # ---- weight + xT loads (first-needed first) ----
            wqk_sb = []
            xt_sb = {}
            for e in range(NE):
                t = w_pool.tile([P, 2 * HDW], bf16, name=f"wqk_sb{e}")
                nc.sync.dma_start(t[:], wqk[e * P : (e + 1) * P, :])
                wqk_sb.append(t)
                t2 = xt_pool.tile([P, SC], bf16, name=f"xt{e}_0", tag=f"xts{e}_0")
                nc.sync.dma_start(t2[:], xT[e * P : (e + 1) * P, 0:SC])
                xt_sb[e, 0] = t2
            wv_sb = []
            for e in range(NE):
                t = w_pool.tile([P, HDW], bf16, name=f"wv_sb{e}")
                nc.sync.dma_start(t[:], wv[e * P : (e + 1) * P, :])
                wv_sb.append(t)
            for sc in range(1, NQC):
                for e in range(NE):
                    t = xt_pool.tile([P, SC], bf16, name=f"xt{e}_{sc}", tag=f"xts{e}_{sc}")
                    nc.sync.dma_start(
                        t[:], xT[e * P : (e + 1) * P, sc * SC : (sc + 1) * SC]
                    )
                    xt_sb[e, sc] = t

            # extended (junk-masked) w_proj tiles, loaded late into recycled
            # xT slots (xt slot (e, sc) frees once phase 1 consumed it)
            wp_sb = {}
            for kt2 in range(2 * NE):
                for nn2 in range(2):
                    t = xt_pool.tile(
                        [P, SC], bf16, name=f"wp{kt2}_{nn2}",
                        tag=f"xts{kt2 % NE}_{(kt2 // NE) * 2 + nn2}",
                    )
                    nc.sync.dma_start(
                        t[:], wp[kt2 * P : (kt2 + 1) * P, nn2 * SC : (nn2 + 1) * SC]
                    )
                    wp_sb[kt2, nn2] = t

            # ---- phase 1: qkT [2*HDW, S] (full-S tiles) and v [S, padded] ----
            qkt_sb = {}
            v_sb = {}
            for fi in range(2):
                qkt_sb[fi] = qkt_pool.tile(
                    [P, S], bf16, name=f"qkt{fi}", tag=f"qktw{fi}"
                )
            # per-head kT with the other head's rows zeroed: score matmuls
            # then run at K=128 (zeros annihilate the foreign q rows), which
            # keeps the PE array utilization high enough for the HAM clock
            # gate to run at full rate
            ktz_sb = {}
            for h in range(HL):
                ktz_sb[h] = qkt_pool.tile([P, S], bf16, name=f"ktz{h}", tag=f"ktz{h}")
                nc.vector.memset(ktz_sb[h][:], 0.0)
            for sc in range(NQC):
                for fi in range(4):
                    ps = at_ps_pool.tile([P, SC], f32, tag="atps", name=f"qk_ps{fi}_{sc}")
                    for e in range(NE):
                        nc.tensor.matmul(
                            ps[:],
                            wqk_sb[e][:, fi * P : (fi + 1) * P],
                            xt_sb[e, sc][:],
                            start=(e == 0),
                            stop=(e == NE - 1),
                        )
                    # fold the 1/sqrt(D)=1/8 score scale into q (bias comes
                    # pre-scaled from the host)
                    if fi < 2:
                        nc.scalar.activation(
                            qkt_sb[fi][:, sc * SC : (sc + 1) * SC],
                            ps[:],
                            Identity,
                            bias=bqk_sb[:, fi : fi + 1],
                            scale=0.125,
                        )
                    else:
                        for hh in range(2):
                            h = 2 * (fi - 2) + hh
                            r0 = 64 * hh
                            nc.scalar.activation(
                                ktz_sb[h][r0 : r0 + D, sc * SC : (sc + 1) * SC],
                                ps[r0 : r0 + D, :],
                                Identity,
                                bias=bqk_sb[r0 : r0 + D, fi : fi + 1],
                            )
                for j in range(4):
                    si = 4 * sc + j
                    psv = wide_ps.tile([P, HDW], f32, tag="wide", name=f"v_ps{si}")
                    for e in range(NE):
                        nc.tensor.matmul(
                            psv[:],
                            xt_sb[e, sc][:, j * P : (j + 1) * P],
                            wv_sb[e][:],
                            start=(e == 0),
                            stop=(e == NE - 1),
                        )
                    # per-head 128-wide slots: [v(64) | ones(1) | zeros(63)]
                    vt = v_pool.tile([P, HL * P], bf16, name=f"v{si}")
                    nc.vector.memset(vt[:], 0.0)
                    nc.sync.dma_start(
                        vt[:].rearrange("p (h u) -> p h u", h=HL)[:, :, D : D + 1],
                        onesb[:],
                    )
                    nc.scalar.activation(
                        vt[:].rearrange("p (h u) -> p h u", h=HL)[:, :, 0:D],
                        psv[:].rearrange("p (h u) -> p h u", h=HL),
                        Identity,
                    )
                    v_sb[si] = vt

            # ---- phase 2: attention, key-block-major per head ----
            # A2A per head-pair kt: chunk j carries heads (2kt, 2kt+1) for
            # s-range j%4; the receiver zeroes the other batch's chunks via wp.
            a2a_in = {}
            a2a_out = {}
            for kt in range(2):
                a2a_in[kt] = dram_pool.tile(
                    [8, P, SC], bf16, tag=f"a2a_in{kt}", name=f"a2a_in{kt}"
                )
                a2a_out[kt] = dram_pool.tile(
                    [8, P, SC], bf16, tag=f"a2a_out{kt}", name=f"a2a_out{kt}"
                )

            def emit_tail(h, qc, at_ps):
                dn_r = small_pool.tile([1, SC], f32r, tag="dnr", name=f"dnr{qc}_{h}")
                nc.vector.tensor_copy(dn_r[:], at_ps[D : D + 1, :])
                # bv folded in as a rank-1 update: (PV + bv denom^T)/denom
                nc.tensor.matmul(
                    at_ps[0:D, :],
                    bv_sb[0:1, h * D : (h + 1) * D],
                    dn_r[:],
                    start=False,
                    stop=True,
                )
                dn32 = small_pool.tile([1, SC], f32, tag="dn32", name=f"dn32{qc}_{h}")
                nc.vector.tensor_copy(dn32[:], at_ps[D : D + 1, :])
                rc32 = small_pool.tile([1, SC], f32, tag="rc32", name=f"rc32{qc}_{h}")
                nc.vector.reciprocal_approx_fast(rc32[:], dn32[:])
                rc = small_pool.tile([1, SC], f32r, tag="rc", name=f"rc{qc}_{h}")
                nc.vector.tensor_copy(rc[:], rc32[:])
                # broadcast 1/denom across partitions via K=1 matmul
                rb = wide_ps.tile([D, SC], f32, tag="wide", name=f"rb{qc}_{h}")
                nc.tensor.matmul(rb[:], ones1[0:1, 0:D], rc[:], start=True, stop=True)
                rb_sb = small_pool.tile([D, SC], f32, tag="rbsb", name=f"rbsb{qc}_{h}")
                nc.vector.tensor_copy(rb_sb[:], rb[:])
                ath = small_pool.tile(
                    [D, SC], bf16, tag="ath", bufs=3, name=f"ath{qc}_{h}"
                )
                nc.vector.tensor_mul(ath[:], at_ps[0:D, :], rb_sb[:])
                kt = h // 2
                prow = 64 * (h % 2)
                nc.sync.dma_start(a2a_in[kt][qc, prow : prow + D, :], ath[:])
                nc.sync.dma_start(a2a_in[kt][qc + 4, prow : prow + D, :], ath[:])

            for h in range(HL):
                fi_q = h // 2
                fi_k = 2 + h // 2
                prow = 64 * (h % 2)
                at_ps = {}
                done = set()
                for qc in range(NQC):
                    at_ps[qc] = at_ps_pool.tile(
                        [P, SC], f32, tag="atps", name=f"at_ps{qc}_{h}"
                    )
                for kb in range(NKB):
                    q0 = P * kb
                    width = S - q0
                    for s2 in range((width + WQ - 1) // WQ):
                        w0 = q0 + s2 * WQ
                        ww = min(WQ, S - w0)
                        scp = wide_ps.tile(
                            [P, WQ], f32, tag="wide", name=f"sc{h}_{kb}_{s2}"
                        )
                        # scoresT[k, q] = kT^T qT (contraction over d, zero-
                        # padded to K=128); matmul output is capped at one
                        # PSUM bank (512 f32)
                        for m0 in range(0, ww, SC):
                            mw = min(SC, ww - m0)
                            nc.tensor.matmul(
                                scp[:, m0 : m0 + mw],
                                ktz_sb[h][:, q0 : q0 + P],
                                qkt_sb[fi_q][:, w0 + m0 : w0 + m0 + mw],
                                start=True,
                                stop=not (s2 == 0 and m0 == 0),
                            )
                        if s2 == 0:
                            # diagonal block: += causal mask via identity
                            # matmul (keeps the scores->exp chain PE-only)
                            nc.tensor.matmul(
                                scp[:, 0:P],
                                id_sb[:],
                                czb_sb[:],
                                start=False,
                                stop=True,
                            )
                        ex = exp_pool.tile(
                            [P, WQ], bf16, tag="exp", name=f"ex{h}_{kb}_{s2}"
                        )
                        nc.scalar.activation(ex[:, 0:ww], scp[:, 0:ww], Exp)
                        # PV pieces per output chunk qc (+denominator via the
                        # ones column of v)
                        qc_lo = w0 // SC
                        qc_hi = (w0 + ww - 1) // SC
                        for qc in range(qc_lo, qc_hi + 1):
                            a0 = max(w0, qc * SC)
                            a1 = min(w0 + ww, (qc + 1) * SC)
                            nc.tensor.matmul(
                                at_ps[qc][:, a0 - qc * SC : a1 - qc * SC],
                                v_sb[kb][:, h * P : (h + 1) * P],
                                ex[:, a0 - w0 : a1 - w0],
                                start=(kb == 0),
                                stop=False,
                            )
                    # deferred normalization: ~2 key-blocks after a chunk's
                    # last contribution, so the PE rarely waits on the DVE
                    # reciprocal chain
                    for qc in range(NQC):
                        if kb == min(4 * qc + 3 + 2, NKB - 1) and qc not in done:
                            done.add(qc)
                            emit_tail(h, qc, at_ps[qc])
                if h % 2 == 1:
                    kt = h // 2
                    nc.gpsimd.collective_compute(
                        "AllToAll",
                        mybir.AluOpType.bypass,
                        ins=[a2a_in[kt][:].opt()],
                        outs=[a2a_out[kt][:].opt()],
                        replica_groups=[list(range(8))],
                    )

            # ---- phase 3: gather + c_proj ----
            # extended-hd row 128*kt2 .. +128 = source core j = kt2//2,
            # head pair kt = kt2%2; other batch's chunks are masked by wp zeros
            lh_sb = {}
            for kt2 in range(2 * NE):
                t = lh_pool.tile([P, SC], bf16, name=f"lh{kt2}")
                nc.sync.dma_start(t[:], a2a_out[kt2 % 2][kt2 // 2])
                lh_sb[kt2] = t
            osb = {}
            # pass 1: even kt2 (head-pair 0, shipped by the first A2A) runs
            # while the second A2A is in flight
            for st in range(4):
                osb[st] = osb_pool.tile(
                    [P, NX], f32, tag=f"osb{st}", name=f"osb{st}", bufs=1
                )
                for nn2 in range(2):
                    pp = wide_ps.tile([P, SC], f32, tag="wide", name=f"pj_e{st}_{nn2}")
                    for i, kt2 in enumerate(range(0, 2 * NE, 2)):
                        nc.tensor.matmul(
                            pp[:],
                            lh_sb[kt2][:, st * P : (st + 1) * P],
                            wp_sb[kt2, nn2][:],
                            start=(i == 0),
                            stop=False,
                        )
                    # + b_proj via K=1 ones matmul (each core owns its rows)
                    nc.tensor.matmul(
                        pp[:],
                        ones1b[0:1, :],
                        bp_sb[0:1, nn2 * SC : (nn2 + 1) * SC],
                        start=False,
                        stop=True,
                    )
                    nc.vector.tensor_copy(osb[st][:, nn2 * SC : (nn2 + 1) * SC], pp[:])
            # pass 2: odd kt2 after the second A2A lands
            for st in range(4):
                for nn2 in range(2):
                    pp = wide_ps.tile([P, SC], f32, tag="wide", name=f"pj_o{st}_{nn2}")
                    for i, kt2 in enumerate(range(1, 2 * NE, 2)):
                        nc.tensor.matmul(
                            pp[:],
                            lh_sb[kt2][:, st * P : (st + 1) * P],
                            wp_sb[kt2, nn2][:],
                            start=(i == 0),
                            stop=(i == NE - 1),
                        )
                    nc.vector.tensor_add(
                        osb[st][:, nn2 * SC : (nn2 + 1) * SC],
                        osb[st][:, nn2 * SC : (nn2 + 1) * SC],
                        pp[:],
                    )
                nc.sync.dma_start(out_ext[st * P : (st + 1) * P, :], osb[st][:])

    nc.compile()
    return nc


def _get_compiled():
    global _COMPILED
    if _COMPILED is None:
        _COMPILED = _build()
    return _COMPILED


def make_in_maps(x, attention_mask, w_attn, b_attn, w_proj, b_proj):
    x = np.asarray(x, dtype=np.float32)
    w_attn = np.asarray(w_attn, dtype=np.float32)
    b_attn = np.asarray(b_attn, dtype=np.float32)
    w_proj = np.asarray(w_proj, dtype=np.float32)
    b_proj = np.asarray(b_proj, dtype=np.float32)

    ki, qi = np.meshgrid(np.arange(P), np.arange(P), indexing="ij")
    causalT = np.where(ki > qi, np.float32(-1e9), np.float32(0.0))
    xTs = [np.ascontiguousarray(x[b].T.astype(BF16)) for b in range(B)]
    # extended w_proj: rows [1024*b : 1024*(b+1)] hold the real w_proj, the
    # other batch's rows are zero (masks that batch's A2A chunks)
    wp_ext = []
    for b in range(B):
        w = np.zeros((2 * NX, NX), dtype=BF16)
        w[NX * b : NX * (b + 1), :] = w_proj.astype(BF16)
        wp_ext.append(w)
    bp_row16 = np.ascontiguousarray(b_proj.reshape(1, NX).astype(BF16))

    in_maps = []
    for c in range(8):
        b, g = divmod(c, 4)
        cols = slice(HDW * g, HDW * (g + 1))
        kcols = slice(NX + HDW * g, NX + HDW * (g + 1))
        vcols = slice(2 * NX + HDW * g, 2 * NX + HDW * (g + 1))
        bqk_arr = np.concatenate([b_attn[cols] * 0.125, b_attn[kcols]]).reshape(4, P)
        in_maps.append(
            {
                "xT": xTs[b],
                "wqk": np.ascontiguousarray(
                    np.concatenate([w_attn[:, cols], w_attn[:, kcols]], axis=1).astype(
                        BF16
                    )
                ),
                "wv": np.ascontiguousarray(w_attn[:, vcols].astype(BF16)),
                "wp": wp_ext[b],
                "bqk": np.ascontiguousarray(bqk_arr),
                "bv": np.ascontiguousarray(b_attn[vcols].reshape(1, HDW)),
                "bp16": bp_row16,
                "identb": np.eye(P, dtype=BF16),
                "causb": causalT.astype(BF16),
                "onesc": np.ones((P, 4), dtype=np.float32),
                "onesb": np.ones((P, 4), dtype=BF16),
            }
        )
    return in_maps


def assemble_out(results):
    out = np.empty((B, S, NX), dtype=np.float32)
    for c in range(8):
        b, g = divmod(c, 4)
        out[b, g * SC : (g + 1) * SC, :] = results[c]["out"]
    return out


def run(in_maps, trace=False):
    from concourse.bass_utils import run_bass_kernel_spmd

    nc = _get_compiled()
    return run_bass_kernel_spmd(nc, in_maps, core_ids=list(range(8)), trace=trace)


def kernel(**inputs) -> np.ndarray:
    in_maps = make_in_maps(**inputs)
    res = run(in_maps)
    return assemble_out(res.results)


if __name__ == "__main__":
    _get_compiled()
    print("build+compile OK")
